# revision 26
# baseline (speedup 1.0000x reference)
"""Causal attentive statistics pooling — Trainium2 Bass kernel (v2).

Strategy (hardcoded for B=8, C=1536, T=4096, A=128, 8 cores):
  - Data-parallel over batch: one sample per NeuronCore.
  - Layout: channels on partitions (12 blocks of 128), time on the free axis.
    Bulk elementwise in bf16 (DVE 2x), prefix ops via tensor_tensor_scan.
  - Key trick: the running mean / running normalized sums are computed with a
    single ratio-recurrence scan  state_t = (d0_t + state_{t-1}) * rho_t
    where rho = count_{t-1}/count_t (resp. Z_{t-1}/Z_t) is an fp32 broadcast
    table.  This emits mean, E[x^2], weighted-mean, and weighted-var directly
    from the scan with no separate [C,T]-sized multiply passes.
  - The causal-mean attention term uses scan(W1m @ (x*m/count_prev)) (matmul
    and column-scaled prefix-sum commute), so mean is never an input to PE.
  - Squares run on ScalarE; sqrt with fused row-sum accumulators produces the
    final std sums; weighted-mean sums come from ScalarE copy+accumulate.
"""

import sys

sys.path.insert(0, "/opt/trn_rl_repo")

from contextlib import ExitStack

import ml_dtypes
import numpy as np

import concourse.bass as bass
import concourse.tile as tile
from concourse import bacc
from concourse import mybir
from concourse.bass_utils import run_bass_kernel_spmd

B, C, T, A = 8, 1536, 4096, 128
P = 128
CB = C // P  # channel blocks
TC = 512  # time chunk
NCH = T // TC
EPS = 1e-12
FW = float(1.0 / (T + EPS))

F32 = mybir.dt.float32
BF16 = mybir.dt.bfloat16
ALU = mybir.AluOpType
ACT = mybir.ActivationFunctionType
BF = ml_dtypes.bfloat16

_CACHE = {}


def build_program():
    FOLD = T // P
    nc = bacc.Bacc("TRN2", target_bir_lowering=False, debug=False)
    scr_d = nc.dram_tensor("zscratch", [1, T], F32)
    scrb_d = nc.dram_tensor("escratch", [1, T], BF16)
    scrb2_d = nc.dram_tensor("etscratch", [1, T], BF16)

    x_d = nc.dram_tensor("x", [C, T], F32, kind="ExternalInput")
    mrc_d = nc.dram_tensor("mrcrow", [1, T], BF16, kind="ExternalInput")
    cp_d = nc.dram_tensor("cprow", [1, T], BF16, kind="ExternalInput")
    rhoc_d = nc.dram_tensor("rhocrow", [1, T], F32, kind="ExternalInput")
    lstrict_d = nc.dram_tensor("lstrict", [P, P], F32, kind="ExternalInput")
    ssub_d = nc.dram_tensor("ssub", [P, P], F32, kind="ExternalInput")
    w1x_d = nc.dram_tensor("w1xT", [C, A], BF16, kind="ExternalInput")
    w1m_d = nc.dram_tensor("w1mT", [C, A], BF16, kind="ExternalInput")
    w1s_d = nc.dram_tensor("w1sT", [C, A], BF16, kind="ExternalInput")
    w2_d = nc.dram_tensor("w2col", [A, 1], BF16, kind="ExternalInput")
    b1_d = nc.dram_tensor("b1col", [A, 1], F32, kind="ExternalInput")
    b2_d = nc.dram_tensor("b2val", [1, 1], F32, kind="ExternalInput")
    out_d = nc.dram_tensor("out", [2, CB, P], F32, kind="ExternalOutput")

    x_r = x_d.rearrange("(k p) t -> p k t", p=P)
    out_r = out_d.rearrange("s k p -> s p k")

    with tile.TileContext(nc) as tc, ExitStack() as ctx:
        const = ctx.enter_context(tc.tile_pool(name="const", bufs=1))
        xpool = ctx.enter_context(tc.tile_pool(name="xpool", bufs=3))
        dbl = ctx.enter_context(tc.tile_pool(name="dbl", bufs=1))
        stdp = ctx.enter_context(tc.tile_pool(name="stdp", bufs=2))
        hot = ctx.enter_context(tc.tile_pool(name="hot", bufs=2))
        psum = ctx.enter_context(tc.tile_pool(name="psum", bufs=2, space="PSUM"))
        psbc = ctx.enter_context(tc.tile_pool(name="psbc", bufs=2, space="PSUM"))

        def bcslc(tbl, t0):
            return (
                tbl[:, t0 : t0 + TC]
                .rearrange("p (o t) -> p o t", o=1)
                .broadcast_to([P, CB, TC])
            )

        # ---- weights / host tables ----
        w1x_sb = const.tile([P, CB, A], BF16)
        w1m_sb = const.tile([P, CB, A], BF16)
        w1s_sb = const.tile([P, CB, A], BF16)
        nc.sync.dma_start(w1x_sb[:], w1x_d.rearrange("(k p) m -> p k m", p=P))
        nc.sync.dma_start(w1m_sb[:], w1m_d.rearrange("(k p) m -> p k m", p=P))
        nc.sync.dma_start(w1s_sb[:], w1s_d.rearrange("(k p) m -> p k m", p=P))
        w2_sb = const.tile([A, 1], BF16)
        b1_sb = const.tile([A, 1], F32)
        b2_sb = const.tile([1, 1], F32)
        nc.sync.dma_start(w2_sb[:], w2_d.ap())
        nc.sync.dma_start(b1_sb[:], b1_d.ap())
        nc.sync.dma_start(b2_sb[:], b2_d.ap())
        lstrict_sb = const.tile([P, P], F32)
        ssub_sb = const.tile([P, P], F32)
        nc.sync.dma_start(lstrict_sb[:], lstrict_d.ap())
        nc.sync.dma_start(ssub_sb[:], ssub_d.ap())

        # broadcast tables: mrcB/cpB bf16; rhoB f32 (shared phase1/phase3)
        mrcB = const.tile([P, T], BF16, tag="tblA")  # m/count_prev, later e~
        cpB = const.tile([P, T], BF16, tag="tblB")  # sqrt(m/count_prev)
        rhoB = const.tile([P, T], F32, tag="tblR")  # rho_c, later rho_z

        # host tables broadcast straight from DRAM (partition-stride-0 DMA)
        nc.sync.dma_start(mrcB[:], mrc_d.ap().broadcast_to([P, T]))
        nc.sync.dma_start(cpB[:], cp_d.ap().broadcast_to([P, T]))
        nc.sync.dma_start(rhoB[:], rhoc_d.ap().broadcast_to([P, T]))

        # carries and accumulators
        meancar = const.tile([P, CB, 1], F32)
        bcar = const.tile([P, CB, 1], F32)
        wmcar = const.tile([P, CB, 1], F32)
        wvcar = const.tile([P, CB, 1], F32)
        ymcar = const.tile([P, 1], F32)
        fm_acc = const.tile([P, CB], F32)
        fs_acc = const.tile([P, CB], F32)
        fm_stage = const.tile([P, CB], F32)
        fs_stage = const.tile([P, CB], F32)
        nc.vector.memset(fs_acc[:], 0.0)
        nc.vector.memset(fm_acc[:], 0.0)

        z_sb = const.tile([P, T], BF16, tag="z")

        # ================= PHASE 1 =================
        for ch in range(NCH):
            t0 = ch * TC

            xbf = xpool.tile([P, CB, TC], BF16, tag="xbf")
            # tiny same-engine write absorbs WAR waits (DMA sync-wait limit)
            nc.gpsimd.memset(xbf[:, :, 0:1], 0.0)
            nc.gpsimd.dma_start(xbf[:], x_r[:, :, t0 : t0 + TC])

            # xt = x * m / count_prev  (mask folded into the table)
            xt = dbl.tile([P, CB, TC], BF16, tag="xt")
            nc.vector.tensor_mul(xt[:], xbf[:], bcslc(mrcB, t0))
            # xxt = (x * sqrt(m/count_prev))^2 = x^2 m / count_prev
            sqx = dbl.tile([P, CB, TC], BF16, tag="sqx")
            nc.vector.tensor_mul(sqx[:], xbf[:], bcslc(cpB, t0))
            nc.scalar.activation(sqx[:], sqx[:], ACT.Square)

            rho2d = rhoB[:, t0 : t0 + TC]
            mean = hot.tile([P, CB, TC], BF16, tag="mean")
            bm2 = hot.tile([P, CB, TC], BF16, tag="b")
            for k in range(CB):
                init = 0.0 if ch == 0 else meancar[:, k, :]
                nc.vector.tensor_tensor_scan(
                    mean[:, k, :], xt[:, k, :], rho2d, init, ALU.add, ALU.mult
                )
            nc.vector.tensor_copy(meancar[:], mean[:, :, TC - 1 : TC])
            for k in range(CB):
                init = 0.0 if ch == 0 else bcar[:, k, :]
                nc.vector.tensor_tensor_scan(
                    bm2[:, k, :], sqx[:, k, :], rho2d, init, ALU.add, ALU.mult
                )
            nc.vector.tensor_copy(bcar[:], bm2[:, :, TC - 1 : TC])

            # var = clamp(b - mean^2), std = sqrt
            mm = dbl.tile([P, CB, TC], BF16, tag="sqx")  # reuse
            nc.scalar.activation(mm[:], mean[:], ACT.Square)
            nc.vector.tensor_sub(bm2[:], bm2[:], mm[:])
            nc.vector.tensor_scalar(bm2[:], bm2[:], EPS, None, ALU.max)
            std = stdp.tile([P, CB, TC], BF16, tag="std")
            nc.scalar.activation(std[:], bm2[:], ACT.Sqrt)

            # PE: zc = W1x @ x + W1s @ std ; ym = W1m @ xt
            zc = psum.tile([P, TC], F32, tag="zc")
            for k in range(CB):
                nc.tensor.matmul(
                    zc[:, :], w1x_sb[:, k, :], xbf[:, k, :],
                    start=(k == 0), stop=False,
                )
            for k in range(CB):
                nc.tensor.matmul(
                    zc[:, :], w1s_sb[:, k, :], std[:, k, :],
                    start=False, stop=(k == CB - 1),
                )
            ym = psum.tile([P, TC], F32, tag="ym")
            for k in range(CB):
                nc.tensor.matmul(
                    ym[:, :], w1m_sb[:, k, :], xt[:, k, :],
                    start=(k == 0), stop=(k == CB - 1),
                )

            # mean-feature: scan(ym; rho_c) directly (column scaling commutes)
            zms = const.tile([P, TC], BF16, tag="zms")
            init = 0.0 if ch == 0 else ymcar[:, :]
            nc.vector.tensor_tensor_scan(
                zms[:, :], ym[:, :], rho2d, init, ALU.add, ALU.mult
            )
            nc.vector.tensor_copy(ymcar[:], zms[:, TC - 1 : TC])
            nc.vector.tensor_add(z_sb[:, t0 : t0 + TC], zc[:, :], zms[:, :])

        # ================= PHASE 2 =================
        nc.scalar.activation(z_sb[:], z_sb[:], ACT.Tanh, bias=b1_sb[:, 0:1], scale=1.0)

        ebf_row = dbl.tile([1, T], BF16, tag="sqx")
        for j in range(T // TC):
            lg = psum.tile([1, TC], F32, tag="zc")
            nc.tensor.matmul(
                lg[:, :], w2_sb[:, :], z_sb[:, j * TC : (j + 1) * TC],
                start=True, stop=True,
            )
            nc.scalar.activation(
                ebf_row[:, j * TC : (j + 1) * TC], lg[:, :], ACT.Exp,
                bias=b2_sb[:, 0:1], scale=1.0,
            )
        # ---- folded Z / reciprocal / ratio pipeline ([128, FOLD]) ----
        scrb_r = scrb_d.rearrange("o (p f) -> (o p) f", p=P)
        scr_r = scr_d.rearrange("o (p f) -> (o p) f", p=P)
        scrb2_r = scrb2_d.rearrange("o (p f) -> (o p) f", p=P)
        nc.sync.dma_start(scrb_d.ap(), ebf_row[:, :])
        efold = const.tile([P, FOLD], BF16, tag="efold")
        nc.sync.dma_start(efold[:], scrb_r)
        zloc = const.tile([P, FOLD], F32, tag="zloc")
        nc.vector.tensor_tensor_scan(
            zloc[:, :], efold[:, :], efold[:, :], 0.0, ALU.add, ALU.bypass
        )
        offp = psbc.tile([P, 1], F32, tag="bc")
        nc.tensor.matmul(
            offp[:, :], lstrict_sb[:, :], zloc[:, FOLD - 1 : FOLD],
            start=True, stop=True,
        )
        offc = const.tile([P, 1], F32, tag="offc")
        nc.scalar.copy(offc[:], offp[:])
        zfold = const.tile([P, FOLD], F32, tag="zfold")
        nc.vector.tensor_scalar(zfold[:], zloc[:], offc[:, 0:1], None, ALU.add)
        rzfold = const.tile([P, FOLD], F32, tag="rzfold")
        nc.vector.reciprocal(rzfold[:], zfold[:])
        # Z_prev (shift by one, seam via sub-diagonal matmul), Z_prev[0] = 1
        seamp = psbc.tile([P, 1], F32, tag="bc")
        nc.tensor.matmul(
            seamp[:, :], ssub_sb[:, :], zfold[:, FOLD - 1 : FOLD],
            start=True, stop=True,
        )
        zpf = const.tile([P, FOLD], F32, tag="zpf")
        nc.vector.tensor_copy(zpf[:, 1:FOLD], zfold[:, 0 : FOLD - 1])
        nc.scalar.copy(zpf[:, 0:1], seamp[:, :])
        nc.vector.memset(zpf[0:1, 0:1], 1.0)
        rzpf = const.tile([P, FOLD], F32, tag="rzpf")
        nc.vector.reciprocal(rzpf[:], zpf[:])
        rhozf = const.tile([P, FOLD], F32, tag="rhozf")
        nc.vector.tensor_mul(rhozf[:], zpf[:], rzfold[:])
        etf = const.tile([P, FOLD], BF16, tag="etf")
        nc.vector.tensor_mul(etf[:], efold[:], rzpf[:])
        # unfold to DRAM, broadcast back into the big tables
        nc.sync.dma_start(scr_r, rhozf[:])
        nc.sync.dma_start(scrb2_r, etf[:])
        etB = const.tile([P, T], BF16, tag="tblA")  # reuse mrcB slot
        nc.sync.dma_start(etB[:], scrb2_d.ap().broadcast_to([P, T]))
        nc.sync.dma_start(rhoB[:], scr_d.ap().broadcast_to([P, T]))

        # ================= PHASE 3 =================
        for ch in range(NCH):
            t0 = ch * TC

            xbf = xpool.tile([P, CB, TC], BF16, tag="xbf")
            nc.gpsimd.memset(xbf[:, :, 0:1], 0.0)
            nc.gpsimd.dma_start(xbf[:], x_r[:, :, t0 : t0 + TC])

            rho2d = rhoB[:, t0 : t0 + TC]

            gt = dbl.tile([P, CB, TC], BF16, tag="xt")
            nc.vector.tensor_mul(gt[:], xbf[:], bcslc(etB, t0))
            wm = hot.tile([P, CB, TC], BF16, tag="mean")
            for k in range(CB):
                init = 0.0 if ch == 0 else wmcar[:, k, :]
                nc.vector.tensor_tensor_scan(
                    wm[:, k, :], gt[:, k, :], rho2d, init, ALU.add, ALU.mult
                )
            nc.vector.tensor_copy(wmcar[:], wm[:, :, TC - 1 : TC])

            # fm partial sums via ScalarE copy+accumulate
            for k in range(CB):
                nc.scalar.activation(
                    gt[:, k, :], wm[:, k, :], ACT.Copy,
                    accum_out=fm_stage[:, k : k + 1],
                )
            nc.vector.tensor_add(fm_acc[:], fm_acc[:], fm_stage[:])

            d = dbl.tile([P, CB, TC], BF16, tag="sqx")
            nc.vector.tensor_sub(d[:], xbf[:], wm[:])
            dd = hot.tile([P, CB, TC], BF16, tag="b")
            nc.scalar.activation(dd[:], d[:], ACT.Square)
            nc.vector.tensor_mul(dd[:], dd[:], bcslc(etB, t0))  # e~ * d^2
            wvar = stdp.tile([P, CB, TC], BF16, tag="std")
            for k in range(CB):
                init = 0.0 if ch == 0 else wvcar[:, k, :]
                nc.vector.tensor_tensor_scan(
                    wvar[:, k, :], dd[:, k, :], rho2d, init, ALU.add, ALU.mult
                )
            nc.vector.tensor_copy(wvcar[:], wvar[:, :, TC - 1 : TC])

            wstd = hot.tile([P, CB, TC], BF16, tag="mean")  # dummy out
            for k in range(CB):
                nc.scalar.activation(
                    wstd[:, k, :], wvar[:, k, :], ACT.Sqrt,
                    accum_out=fs_stage[:, k : k + 1],
                )
            nc.vector.tensor_add(fs_acc[:], fs_acc[:], fs_stage[:])

        # ================= FINALIZE =================
        nc.vector.tensor_scalar(fm_acc[:], fm_acc[:], FW, None, ALU.mult)
        nc.vector.tensor_scalar(fs_acc[:], fs_acc[:], FW, None, ALU.mult)
        nc.sync.dma_start(out_r[0], fm_acc[:])
        nc.sync.dma_start(out_r[1], fs_acc[:])

    nc.finalize()
    return nc


def _get_program():
    if "nc" not in _CACHE:
        _CACHE["nc"] = build_program()
    return _CACHE["nc"]


def host_tables(ln, Tdim):
    """Per-sample tables: m/count_prev (bf16), count_prev (bf16),
    count_prev/count (f32)."""
    t = np.arange(Tdim)
    m = (t < ln).astype(np.float64)
    count = np.clip(np.cumsum(m), 1.0, None)
    cprev = np.concatenate([[1.0], count[:-1]])
    mrc = (m / cprev).astype(BF).reshape(1, Tdim)
    cp = np.sqrt(m / cprev).astype(BF).reshape(1, Tdim)
    rhoc = (cprev / count).astype(np.float32).reshape(1, Tdim)
    return mrc, cp, rhoc


_LSTRICT = (np.tril(np.ones((P, P)), -1) - np.tril(np.ones((P, P)), -1).T * 0).astype(np.float32).T.copy()
_SSUB = np.zeros((P, P), np.float32)
for _i in range(1, P):
    _SSUB[_i - 1, _i] = 1.0


def make_in_map(xb, ln, W1, b1, W2, b2, Cdim, Tdim):
    mrc, cp, rhoc = host_tables(ln, Tdim)
    return {
        "lstrict": _LSTRICT,
        "ssub": _SSUB,
        "x": np.ascontiguousarray(xb),
        "mrcrow": mrc,
        "cprow": cp,
        "rhocrow": rhoc,
        "w1xT": np.ascontiguousarray(W1[:, 0:Cdim].T).astype(BF),
        "w1mT": np.ascontiguousarray(W1[:, Cdim : 2 * Cdim].T).astype(BF),
        "w1sT": np.ascontiguousarray(W1[:, 2 * Cdim : 3 * Cdim].T).astype(BF),
        "w2col": np.ascontiguousarray(W2.T).astype(BF),
        "b1col": b1.reshape(A, 1).astype(np.float32),
        "b2val": b2.reshape(1, 1).astype(np.float32),
    }


def kernel(x, lengths, W1, b1, W2, b2):
    x = np.asarray(x, dtype=np.float32)
    lengths = np.asarray(lengths)
    W1 = np.asarray(W1, dtype=np.float32)
    b1 = np.asarray(b1, dtype=np.float32)
    W2 = np.asarray(W2, dtype=np.float32)
    b2 = np.asarray(b2, dtype=np.float32)

    nc = _get_program()
    in_maps = [
        make_in_map(x[b], int(lengths[b]), W1, b1, W2, b2, C, T) for b in range(B)
    ]

    import os

    trace = bool(os.environ.get("BASS_KERNEL_TRACE"))
    try:
        res = run_bass_kernel_spmd(nc, in_maps, core_ids=list(range(B)), trace=trace)
    except Exception:
        # transient device errors have been observed; retry once
        import time as _time

        _time.sleep(2.0)
        res = run_bass_kernel_spmd(nc, in_maps, core_ids=list(range(B)), trace=trace)
    _CACHE["exec_time_ns"] = getattr(res, "exec_time_ns", None)
    _CACHE["results_obj"] = res

    outs = []
    for b in range(B):
        o = np.asarray(res.results[b]["out"], dtype=np.float32)
        outs.append(np.concatenate([o[0].reshape(C), o[1].reshape(C)]))
    return np.stack(outs).astype(np.float32)


# revision 27
# speedup vs baseline: 1.1191x; 1.1191x over previous
"""Causal attentive statistics pooling — Trainium2 Bass kernel (v2).

Strategy (hardcoded for B=8, C=1536, T=4096, A=128, 8 cores):
  - Data-parallel over batch: one sample per NeuronCore.
  - Layout: channels on partitions (12 blocks of 128), time on the free axis.
    Bulk elementwise in bf16 (DVE 2x), prefix ops via tensor_tensor_scan.
  - Key trick: the running mean / running normalized sums are computed with a
    single ratio-recurrence scan  state_t = (d0_t + state_{t-1}) * rho_t
    where rho = count_{t-1}/count_t (resp. Z_{t-1}/Z_t) is an fp32 broadcast
    table.  This emits mean, E[x^2], weighted-mean, and weighted-var directly
    from the scan with no separate [C,T]-sized multiply passes.
  - The causal-mean attention term uses scan(W1m @ (x*m/count_prev)) (matmul
    and column-scaled prefix-sum commute), so mean is never an input to PE.
  - Squares run on ScalarE; sqrt with fused row-sum accumulators produces the
    final std sums; weighted-mean sums come from ScalarE copy+accumulate.
"""

import sys

sys.path.insert(0, "/opt/trn_rl_repo")

from contextlib import ExitStack

import ml_dtypes
import numpy as np

import concourse.bass as bass
import concourse.tile as tile
from concourse import bacc
from concourse import mybir
from concourse.bass_utils import run_bass_kernel_spmd

B, C, T, A = 8, 1536, 4096, 128
P = 128
CB = C // P  # channel blocks
TC = 512  # time chunk
NCH = T // TC
EPS = 1e-12
FW = float(1.0 / (T + EPS))

F32 = mybir.dt.float32
BF16 = mybir.dt.bfloat16
ALU = mybir.AluOpType
ACT = mybir.ActivationFunctionType
BF = ml_dtypes.bfloat16

_CACHE = {}


def build_program():
    FOLD = T // P
    nc = bacc.Bacc("TRN2", target_bir_lowering=False, debug=False)
    scr_d = nc.dram_tensor("zscratch", [1, T], F32)
    scrb_d = nc.dram_tensor("escratch", [1, T], BF16)
    scrb2_d = nc.dram_tensor("etscratch", [1, T], BF16)

    x_d = nc.dram_tensor("x", [C, T], F32, kind="ExternalInput")
    mrc_d = nc.dram_tensor("mrcrow", [1, T], BF16, kind="ExternalInput")
    cp_d = nc.dram_tensor("cprow", [1, T], BF16, kind="ExternalInput")
    rhoc_d = nc.dram_tensor("rhocrow", [1, T], F32, kind="ExternalInput")
    lstrict_d = nc.dram_tensor("lstrict", [P, P], F32, kind="ExternalInput")
    ssub_d = nc.dram_tensor("ssub", [P, P], F32, kind="ExternalInput")
    w1x_d = nc.dram_tensor("w1xT", [C, A], BF16, kind="ExternalInput")
    w1m_d = nc.dram_tensor("w1mT", [C, A], BF16, kind="ExternalInput")
    w1s_d = nc.dram_tensor("w1sT", [C, A], BF16, kind="ExternalInput")
    w2_d = nc.dram_tensor("w2col", [A, 1], BF16, kind="ExternalInput")
    b1_d = nc.dram_tensor("b1col", [A, 1], F32, kind="ExternalInput")
    b2_d = nc.dram_tensor("b2val", [1, 1], F32, kind="ExternalInput")
    out_d = nc.dram_tensor("out", [2, CB, P], F32, kind="ExternalOutput")

    x_r = x_d.rearrange("(k p) t -> p k t", p=P)
    out_r = out_d.rearrange("s k p -> s p k")

    with tile.TileContext(nc) as tc, ExitStack() as ctx:
        const = ctx.enter_context(tc.tile_pool(name="const", bufs=1))
        xpool = ctx.enter_context(tc.tile_pool(name="xpool", bufs=3))
        dbl = ctx.enter_context(tc.tile_pool(name="dbl", bufs=1))
        stdp = ctx.enter_context(tc.tile_pool(name="stdp", bufs=2))
        hot = ctx.enter_context(tc.tile_pool(name="hot", bufs=2))
        psum = ctx.enter_context(tc.tile_pool(name="psum", bufs=2, space="PSUM"))
        psbc = ctx.enter_context(tc.tile_pool(name="psbc", bufs=2, space="PSUM"))

        def bcslc(tbl, t0):
            return (
                tbl[:, t0 : t0 + TC]
                .rearrange("p (o t) -> p o t", o=1)
                .broadcast_to([P, CB, TC])
            )

        # ---- weights / host tables ----
        w1x_sb = const.tile([P, CB, A], BF16)
        w1m_sb = const.tile([P, CB, A], BF16)
        w1s_sb = const.tile([P, CB, A], BF16)
        nc.sync.dma_start(w1x_sb[:], w1x_d.rearrange("(k p) m -> p k m", p=P))
        nc.sync.dma_start(w1m_sb[:], w1m_d.rearrange("(k p) m -> p k m", p=P))
        nc.sync.dma_start(w1s_sb[:], w1s_d.rearrange("(k p) m -> p k m", p=P))
        w2_sb = const.tile([A, 1], BF16)
        b1_sb = const.tile([A, 1], F32)
        b2_sb = const.tile([1, 1], F32)
        nc.sync.dma_start(w2_sb[:], w2_d.ap())
        nc.sync.dma_start(b1_sb[:], b1_d.ap())
        nc.sync.dma_start(b2_sb[:], b2_d.ap())
        lstrict_sb = const.tile([P, P], F32)
        ssub_sb = const.tile([P, P], F32)
        nc.sync.dma_start(lstrict_sb[:], lstrict_d.ap())
        nc.sync.dma_start(ssub_sb[:], ssub_d.ap())

        # broadcast tables: mrcB/cpB bf16; rhoB f32 (shared phase1/phase3)
        mrcB = const.tile([P, T], BF16, tag="tblA")  # m/count_prev, later e~
        cpB = const.tile([P, T], BF16, tag="tblB")  # count_prev
        rhoB = const.tile([P, T], F32, tag="tblR")  # rho_c, later rho_z

        # host tables broadcast straight from DRAM (partition-stride-0 DMA)
        nc.sync.dma_start(mrcB[:], mrc_d.ap().broadcast_to([P, T]))
        nc.sync.dma_start(cpB[:], cp_d.ap().broadcast_to([P, T]))
        nc.sync.dma_start(rhoB[:], rhoc_d.ap().broadcast_to([P, T]))

        # carries and accumulators
        meancar = const.tile([P, CB, 1], F32)
        bcar = const.tile([P, CB, 1], F32)
        wmcar = const.tile([P, CB, 1], F32)
        wvcar = const.tile([P, CB, 1], F32)
        ymcar = const.tile([P, 1], F32)
        fm_acc = const.tile([P, CB], F32)
        fs_acc = const.tile([P, CB], F32)
        fm_stage = const.tile([P, CB], F32)
        fs_stage = const.tile([P, CB], F32)
        nc.vector.memset(fs_acc[:], 0.0)
        nc.vector.memset(fm_acc[:], 0.0)

        z_sb = const.tile([P, T], BF16, tag="z")

        # ================= PHASE 1 =================
        for ch in range(NCH):
            t0 = ch * TC

            xbf = xpool.tile([P, CB, TC], BF16, tag="xbf")
            # tiny same-engine write absorbs WAR waits (DMA sync-wait limit)
            nc.gpsimd.memset(xbf[:, :, 0:1], 0.0)
            nc.gpsimd.dma_start(xbf[:], x_r[:, :, t0 : t0 + TC])

            # xt = x * m / count_prev  (mask folded into the table)
            xt = dbl.tile([P, CB, TC], BF16, tag="xt")
            nc.vector.tensor_mul(xt[:], xbf[:], bcslc(mrcB, t0))
            # xxt = xt^2 * count_prev = x^2 m / count_prev
            sqx = dbl.tile([P, CB, TC], BF16, tag="sqx")
            nc.scalar.activation(sqx[:], xt[:], ACT.Square)
            nc.vector.tensor_mul(sqx[:], sqx[:], bcslc(cpB, t0))

            rho2d = rhoB[:, t0 : t0 + TC]
            mean = hot.tile([P, CB, TC], BF16, tag="mean")
            bm2 = hot.tile([P, CB, TC], BF16, tag="b")
            for k in range(CB):
                init = 0.0 if ch == 0 else meancar[:, k, :]
                nc.vector.tensor_tensor_scan(
                    mean[:, k, :], xt[:, k, :], rho2d, init, ALU.add, ALU.mult
                )
            nc.vector.tensor_copy(meancar[:], mean[:, :, TC - 1 : TC])
            for k in range(CB):
                init = 0.0 if ch == 0 else bcar[:, k, :]
                nc.vector.tensor_tensor_scan(
                    bm2[:, k, :], sqx[:, k, :], rho2d, init, ALU.add, ALU.mult
                )
            nc.vector.tensor_copy(bcar[:], bm2[:, :, TC - 1 : TC])

            # var = clamp(b - mean^2), std = sqrt
            mm = dbl.tile([P, CB, TC], BF16, tag="sqx")  # reuse
            nc.scalar.activation(mm[:], mean[:], ACT.Square)
            nc.vector.tensor_sub(bm2[:], bm2[:], mm[:])
            nc.vector.tensor_scalar(bm2[:], bm2[:], EPS, None, ALU.max)
            std = stdp.tile([P, CB, TC], BF16, tag="std")
            nc.scalar.activation(std[:], bm2[:], ACT.Sqrt)

            # PE: zc = W1x @ x + W1s @ std ; ym = W1m @ xt
            zc = psum.tile([P, TC], F32, tag="zc")
            for k in range(CB):
                nc.tensor.matmul(
                    zc[:, :], w1x_sb[:, k, :], xbf[:, k, :],
                    start=(k == 0), stop=False,
                )
            for k in range(CB):
                nc.tensor.matmul(
                    zc[:, :], w1s_sb[:, k, :], std[:, k, :],
                    start=False, stop=(k == CB - 1),
                )
            ym = psum.tile([P, TC], F32, tag="ym")
            for k in range(CB):
                nc.tensor.matmul(
                    ym[:, :], w1m_sb[:, k, :], xt[:, k, :],
                    start=(k == 0), stop=(k == CB - 1),
                )

            # mean-feature: scan(ym; rho_c) directly (column scaling commutes)
            zms = const.tile([P, TC], BF16, tag="zms")
            init = 0.0 if ch == 0 else ymcar[:, :]
            nc.vector.tensor_tensor_scan(
                zms[:, :], ym[:, :], rho2d, init, ALU.add, ALU.mult
            )
            nc.vector.tensor_copy(ymcar[:], zms[:, TC - 1 : TC])
            nc.vector.tensor_add(z_sb[:, t0 : t0 + TC], zc[:, :], zms[:, :])

        # ================= PHASE 2 =================
        nc.scalar.activation(z_sb[:], z_sb[:], ACT.Tanh, bias=b1_sb[:, 0:1], scale=1.0)

        ebf_row = dbl.tile([1, T], BF16, tag="sqx")
        for j in range(T // TC):
            lg = psum.tile([1, TC], F32, tag="zc")
            nc.tensor.matmul(
                lg[:, :], w2_sb[:, :], z_sb[:, j * TC : (j + 1) * TC],
                start=True, stop=True,
            )
            nc.scalar.activation(
                ebf_row[:, j * TC : (j + 1) * TC], lg[:, :], ACT.Exp,
                bias=b2_sb[:, 0:1], scale=1.0,
            )
        # ---- folded Z / reciprocal / ratio pipeline ([128, FOLD]) ----
        scrb_r = scrb_d.rearrange("o (p f) -> (o p) f", p=P)
        scr_r = scr_d.rearrange("o (p f) -> (o p) f", p=P)
        scrb2_r = scrb2_d.rearrange("o (p f) -> (o p) f", p=P)
        nc.sync.dma_start(scrb_d.ap(), ebf_row[:, :])
        efold = const.tile([P, FOLD], BF16, tag="efold")
        nc.sync.dma_start(efold[:], scrb_r)
        zloc = const.tile([P, FOLD], F32, tag="zloc")
        nc.vector.tensor_tensor_scan(
            zloc[:, :], efold[:, :], efold[:, :], 0.0, ALU.add, ALU.bypass
        )
        offp = psbc.tile([P, 1], F32, tag="bc")
        nc.tensor.matmul(
            offp[:, :], lstrict_sb[:, :], zloc[:, FOLD - 1 : FOLD],
            start=True, stop=True,
        )
        offc = const.tile([P, 1], F32, tag="offc")
        nc.scalar.copy(offc[:], offp[:])
        zfold = const.tile([P, FOLD], F32, tag="zfold")
        nc.vector.tensor_scalar(zfold[:], zloc[:], offc[:, 0:1], None, ALU.add)
        rzfold = const.tile([P, FOLD], F32, tag="rzfold")
        nc.vector.reciprocal(rzfold[:], zfold[:])
        # Z_prev (shift by one, seam via sub-diagonal matmul), Z_prev[0] = 1
        seamp = psbc.tile([P, 1], F32, tag="bc")
        nc.tensor.matmul(
            seamp[:, :], ssub_sb[:, :], zfold[:, FOLD - 1 : FOLD],
            start=True, stop=True,
        )
        zpf = const.tile([P, FOLD], F32, tag="zpf")
        nc.vector.tensor_copy(zpf[:, 1:FOLD], zfold[:, 0 : FOLD - 1])
        nc.scalar.copy(zpf[:, 0:1], seamp[:, :])
        nc.vector.memset(zpf[0:1, 0:1], 1.0)
        rzpf = const.tile([P, FOLD], F32, tag="rzpf")
        nc.vector.reciprocal(rzpf[:], zpf[:])
        rhozf = const.tile([P, FOLD], F32, tag="rhozf")
        nc.vector.tensor_mul(rhozf[:], zpf[:], rzfold[:])
        etf = const.tile([P, FOLD], BF16, tag="etf")
        nc.vector.tensor_mul(etf[:], efold[:], rzpf[:])
        # unfold to DRAM, broadcast back into the big tables
        nc.sync.dma_start(scr_r, rhozf[:])
        nc.sync.dma_start(scrb2_r, etf[:])
        etB = const.tile([P, T], BF16, tag="tblA")  # reuse mrcB slot
        nc.sync.dma_start(etB[:], scrb2_d.ap().broadcast_to([P, T]))
        nc.sync.dma_start(rhoB[:], scr_d.ap().broadcast_to([P, T]))

        # ================= PHASE 3 =================
        for ch in range(NCH):
            t0 = ch * TC

            xbf = xpool.tile([P, CB, TC], BF16, tag="xbf")
            nc.gpsimd.memset(xbf[:, :, 0:1], 0.0)
            nc.gpsimd.dma_start(xbf[:], x_r[:, :, t0 : t0 + TC])

            rho2d = rhoB[:, t0 : t0 + TC]

            gt = dbl.tile([P, CB, TC], BF16, tag="xt")
            nc.vector.tensor_mul(gt[:], xbf[:], bcslc(etB, t0))
            wm = hot.tile([P, CB, TC], BF16, tag="mean")
            for k in range(CB):
                init = 0.0 if ch == 0 else wmcar[:, k, :]
                nc.vector.tensor_tensor_scan(
                    wm[:, k, :], gt[:, k, :], rho2d, init, ALU.add, ALU.mult
                )
            nc.vector.tensor_copy(wmcar[:], wm[:, :, TC - 1 : TC])

            # fm partial sums via ScalarE copy+accumulate
            for k in range(CB):
                nc.scalar.activation(
                    gt[:, k, :], wm[:, k, :], ACT.Copy,
                    accum_out=fm_stage[:, k : k + 1],
                )
            nc.vector.tensor_add(fm_acc[:], fm_acc[:], fm_stage[:])

            d = dbl.tile([P, CB, TC], BF16, tag="sqx")
            nc.vector.tensor_sub(d[:], xbf[:], wm[:])
            dd = hot.tile([P, CB, TC], BF16, tag="b")
            nc.scalar.activation(dd[:], d[:], ACT.Square)
            nc.vector.tensor_mul(dd[:], dd[:], bcslc(etB, t0))  # e~ * d^2
            wvar = stdp.tile([P, CB, TC], BF16, tag="std")
            for k in range(CB):
                init = 0.0 if ch == 0 else wvcar[:, k, :]
                nc.vector.tensor_tensor_scan(
                    wvar[:, k, :], dd[:, k, :], rho2d, init, ALU.add, ALU.mult
                )
            nc.vector.tensor_copy(wvcar[:], wvar[:, :, TC - 1 : TC])

            wstd = hot.tile([P, CB, TC], BF16, tag="mean")  # dummy out
            for k in range(CB):
                nc.scalar.activation(
                    wstd[:, k, :], wvar[:, k, :], ACT.Sqrt,
                    accum_out=fs_stage[:, k : k + 1],
                )
            nc.vector.tensor_add(fs_acc[:], fs_acc[:], fs_stage[:])

        # ================= FINALIZE =================
        nc.vector.tensor_scalar(fm_acc[:], fm_acc[:], FW, None, ALU.mult)
        nc.vector.tensor_scalar(fs_acc[:], fs_acc[:], FW, None, ALU.mult)
        nc.sync.dma_start(out_r[0], fm_acc[:])
        nc.sync.dma_start(out_r[1], fs_acc[:])

    nc.finalize()
    return nc


def _get_program():
    if "nc" not in _CACHE:
        _CACHE["nc"] = build_program()
    return _CACHE["nc"]


def host_tables(ln, Tdim):
    """Per-sample tables: m/count_prev (bf16), count_prev (bf16),
    count_prev/count (f32)."""
    t = np.arange(Tdim)
    m = (t < ln).astype(np.float64)
    count = np.clip(np.cumsum(m), 1.0, None)
    cprev = np.concatenate([[1.0], count[:-1]])
    mrc = (m / cprev).astype(BF).reshape(1, Tdim)
    cp = cprev.astype(BF).reshape(1, Tdim)
    rhoc = (cprev / count).astype(np.float32).reshape(1, Tdim)
    return mrc, cp, rhoc


_LSTRICT = (np.tril(np.ones((P, P)), -1) - np.tril(np.ones((P, P)), -1).T * 0).astype(np.float32).T.copy()
_SSUB = np.zeros((P, P), np.float32)
for _i in range(1, P):
    _SSUB[_i - 1, _i] = 1.0


def make_in_map(xb, ln, W1, b1, W2, b2, Cdim, Tdim):
    mrc, cp, rhoc = host_tables(ln, Tdim)
    return {
        "lstrict": _LSTRICT,
        "ssub": _SSUB,
        "x": np.ascontiguousarray(xb),
        "mrcrow": mrc,
        "cprow": cp,
        "rhocrow": rhoc,
        "w1xT": np.ascontiguousarray(W1[:, 0:Cdim].T).astype(BF),
        "w1mT": np.ascontiguousarray(W1[:, Cdim : 2 * Cdim].T).astype(BF),
        "w1sT": np.ascontiguousarray(W1[:, 2 * Cdim : 3 * Cdim].T).astype(BF),
        "w2col": np.ascontiguousarray(W2.T).astype(BF),
        "b1col": b1.reshape(A, 1).astype(np.float32),
        "b2val": b2.reshape(1, 1).astype(np.float32),
    }


def kernel(x, lengths, W1, b1, W2, b2):
    x = np.asarray(x, dtype=np.float32)
    lengths = np.asarray(lengths)
    W1 = np.asarray(W1, dtype=np.float32)
    b1 = np.asarray(b1, dtype=np.float32)
    W2 = np.asarray(W2, dtype=np.float32)
    b2 = np.asarray(b2, dtype=np.float32)

    nc = _get_program()
    in_maps = [
        make_in_map(x[b], int(lengths[b]), W1, b1, W2, b2, C, T) for b in range(B)
    ]

    import os

    trace = bool(os.environ.get("BASS_KERNEL_TRACE"))
    try:
        res = run_bass_kernel_spmd(nc, in_maps, core_ids=list(range(B)), trace=trace)
    except Exception:
        # transient device errors have been observed; retry once
        import time as _time

        _time.sleep(2.0)
        res = run_bass_kernel_spmd(nc, in_maps, core_ids=list(range(B)), trace=trace)
    _CACHE["exec_time_ns"] = getattr(res, "exec_time_ns", None)
    _CACHE["results_obj"] = res

    outs = []
    for b in range(B):
        o = np.asarray(res.results[b]["out"], dtype=np.float32)
        outs.append(np.concatenate([o[0].reshape(C), o[1].reshape(C)]))
    return np.stack(outs).astype(np.float32)


# revision 28
# speedup vs baseline: 1.1203x; 1.0010x over previous
"""Causal attentive statistics pooling — Trainium2 Bass kernel (v2).

Strategy (hardcoded for B=8, C=1536, T=4096, A=128, 8 cores):
  - Data-parallel over batch: one sample per NeuronCore.
  - Layout: channels on partitions (12 blocks of 128), time on the free axis.
    Bulk elementwise in bf16 (DVE 2x), prefix ops via tensor_tensor_scan.
  - Key trick: the running mean / running normalized sums are computed with a
    single ratio-recurrence scan  state_t = (d0_t + state_{t-1}) * rho_t
    where rho = count_{t-1}/count_t (resp. Z_{t-1}/Z_t) is an fp32 broadcast
    table.  This emits mean, E[x^2], weighted-mean, and weighted-var directly
    from the scan with no separate [C,T]-sized multiply passes.
  - The causal-mean attention term uses scan(W1m @ (x*m/count_prev)) (matmul
    and column-scaled prefix-sum commute), so mean is never an input to PE.
  - Squares run on ScalarE; sqrt with fused row-sum accumulators produces the
    final std sums; weighted-mean sums come from ScalarE copy+accumulate.
"""

import sys

sys.path.insert(0, "/opt/trn_rl_repo")

from contextlib import ExitStack

import ml_dtypes
import numpy as np

import concourse.bass as bass
import concourse.tile as tile
from concourse import bacc
from concourse import mybir
from concourse.bass_utils import run_bass_kernel_spmd

B, C, T, A = 8, 1536, 4096, 128
P = 128
CB = C // P  # channel blocks
TC = 512  # time chunk
NCH = T // TC
EPS = 1e-12
FW = float(1.0 / (T + EPS))

F32 = mybir.dt.float32
BF16 = mybir.dt.bfloat16
ALU = mybir.AluOpType
ACT = mybir.ActivationFunctionType
BF = ml_dtypes.bfloat16

_CACHE = {}


def build_program():
    FOLD = T // P
    nc = bacc.Bacc("TRN2", target_bir_lowering=False, debug=False)
    scr_d = nc.dram_tensor("zscratch", [1, T], F32)
    scrb_d = nc.dram_tensor("escratch", [1, T], BF16)
    scrb2_d = nc.dram_tensor("etscratch", [1, T], BF16)

    x_d = nc.dram_tensor("x", [C, T], F32, kind="ExternalInput")
    mrc_d = nc.dram_tensor("mrcrow", [1, T], BF16, kind="ExternalInput")
    cp_d = nc.dram_tensor("cprow", [1, T], BF16, kind="ExternalInput")
    rhoc_d = nc.dram_tensor("rhocrow", [1, T], F32, kind="ExternalInput")
    lstrict_d = nc.dram_tensor("lstrict", [P, P], F32, kind="ExternalInput")
    ssub_d = nc.dram_tensor("ssub", [P, P], F32, kind="ExternalInput")
    w1x_d = nc.dram_tensor("w1xT", [C, A], BF16, kind="ExternalInput")
    w1m_d = nc.dram_tensor("w1mT", [C, A], BF16, kind="ExternalInput")
    w1s_d = nc.dram_tensor("w1sT", [C, A], BF16, kind="ExternalInput")
    w2_d = nc.dram_tensor("w2col", [A, 1], BF16, kind="ExternalInput")
    b1_d = nc.dram_tensor("b1col", [A, 1], F32, kind="ExternalInput")
    b2_d = nc.dram_tensor("b2val", [1, 1], F32, kind="ExternalInput")
    out_d = nc.dram_tensor("out", [2, CB, P], F32, kind="ExternalOutput")

    x_r = x_d.rearrange("(k p) t -> p k t", p=P)
    out_r = out_d.rearrange("s k p -> s p k")

    with tile.TileContext(nc) as tc, ExitStack() as ctx:
        const = ctx.enter_context(tc.tile_pool(name="const", bufs=1))
        xpool = ctx.enter_context(tc.tile_pool(name="xpool", bufs=3))
        dbl = ctx.enter_context(tc.tile_pool(name="dbl", bufs=1))
        stdp = ctx.enter_context(tc.tile_pool(name="stdp", bufs=2))
        hot = ctx.enter_context(tc.tile_pool(name="hot", bufs=2))
        psum = ctx.enter_context(tc.tile_pool(name="psum", bufs=2, space="PSUM"))
        psbc = ctx.enter_context(tc.tile_pool(name="psbc", bufs=2, space="PSUM"))

        def bcslc(tbl, t0):
            return (
                tbl[:, t0 : t0 + TC]
                .rearrange("p (o t) -> p o t", o=1)
                .broadcast_to([P, CB, TC])
            )

        # ---- weights / host tables ----
        w1x_sb = const.tile([P, CB, A], BF16)
        w1m_sb = const.tile([P, CB, A], BF16)
        w1s_sb = const.tile([P, CB, A], BF16)
        nc.sync.dma_start(w1x_sb[:], w1x_d.rearrange("(k p) m -> p k m", p=P))
        nc.sync.dma_start(w1m_sb[:], w1m_d.rearrange("(k p) m -> p k m", p=P))
        nc.sync.dma_start(w1s_sb[:], w1s_d.rearrange("(k p) m -> p k m", p=P))
        w2_sb = const.tile([A, 1], BF16)
        b1_sb = const.tile([A, 1], F32)
        b2_sb = const.tile([1, 1], F32)
        nc.sync.dma_start(w2_sb[:], w2_d.ap())
        nc.sync.dma_start(b1_sb[:], b1_d.ap())
        nc.sync.dma_start(b2_sb[:], b2_d.ap())
        lstrict_sb = const.tile([P, P], F32)
        ssub_sb = const.tile([P, P], F32)
        nc.sync.dma_start(lstrict_sb[:], lstrict_d.ap())
        nc.sync.dma_start(ssub_sb[:], ssub_d.ap())

        # broadcast tables: mrcB/cpB bf16; rhoB f32 (shared phase1/phase3)
        mrcB = const.tile([P, T], BF16, tag="tblA")  # m/count_prev, later e~
        cpB = const.tile([P, T], BF16, tag="tblB")  # count_prev
        rhoB = const.tile([P, T], F32, tag="tblR")  # rho_c, later rho_z

        # host tables broadcast straight from DRAM (partition-stride-0 DMA)
        nc.sync.dma_start(mrcB[:], mrc_d.ap().broadcast_to([P, T]))
        nc.sync.dma_start(cpB[:], cp_d.ap().broadcast_to([P, T]))
        nc.sync.dma_start(rhoB[:], rhoc_d.ap().broadcast_to([P, T]))

        # carries and accumulators
        meancar = const.tile([P, CB, 1], F32)
        bcar = const.tile([P, CB, 1], F32)
        wmcar = const.tile([P, CB, 1], F32)
        wvcar = const.tile([P, CB, 1], F32)
        ymcar = const.tile([P, 1], F32)
        fm_acc = const.tile([P, CB], F32)
        fs_acc = const.tile([P, CB], F32)
        fm_stage = const.tile([P, CB], F32)
        fs_stage = const.tile([P, CB], F32)
        nc.vector.memset(fs_acc[:], 0.0)
        nc.vector.memset(fm_acc[:], 0.0)

        z_sb = const.tile([P, T], BF16, tag="z")

        # ================= PHASE 1 =================
        for ch in range(NCH):
            t0 = ch * TC

            xbf = xpool.tile([P, CB, TC], BF16, tag="xbf")
            # tiny same-engine write absorbs WAR waits (DMA sync-wait limit)
            nc.gpsimd.memset(xbf[:, :, 0:1], 0.0)
            nc.gpsimd.dma_start(xbf[:], x_r[:, :, t0 : t0 + TC])

            # xt = x * m / count_prev  (mask folded into the table)
            xt = dbl.tile([P, CB, TC], BF16, tag="xt")
            nc.vector.tensor_mul(xt[:], xbf[:], bcslc(mrcB, t0))
            # xxt = xt^2 * count_prev = x^2 m / count_prev
            sqx = dbl.tile([P, CB, TC], BF16, tag="sqx")
            nc.scalar.activation(sqx[:], xt[:], ACT.Square)
            nc.vector.tensor_mul(sqx[:], sqx[:], bcslc(cpB, t0))

            rho2d = rhoB[:, t0 : t0 + TC]
            mean = hot.tile([P, CB, TC], BF16, tag="mean")
            bm2 = hot.tile([P, CB, TC], BF16, tag="b")
            for k in range(CB):
                init = 0.0 if ch == 0 else meancar[:, k, :]
                nc.vector.tensor_tensor_scan(
                    mean[:, k, :], xt[:, k, :], rho2d, init, ALU.add, ALU.mult
                )
            nc.vector.tensor_copy(meancar[:], mean[:, :, TC - 1 : TC])
            for k in range(CB):
                init = 0.0 if ch == 0 else bcar[:, k, :]
                nc.vector.tensor_tensor_scan(
                    bm2[:, k, :], sqx[:, k, :], rho2d, init, ALU.add, ALU.mult
                )
            nc.vector.tensor_copy(bcar[:], bm2[:, :, TC - 1 : TC])

            # var = clamp(b - mean^2), std = sqrt
            mm = dbl.tile([P, CB, TC], BF16, tag="sqx")  # reuse
            nc.scalar.activation(mm[:], mean[:], ACT.Square)
            nc.vector.tensor_sub(bm2[:], bm2[:], mm[:])
            nc.vector.tensor_scalar(bm2[:], bm2[:], EPS, None, ALU.max)
            std = stdp.tile([P, CB, TC], BF16, tag="std")
            nc.scalar.activation(std[:], bm2[:], ACT.Sqrt)

            # PE: zc = W1x @ x + W1s @ std ; ym = W1m @ xt
            zc = psum.tile([P, TC], F32, tag="zc")
            for k in range(CB):
                nc.tensor.matmul(
                    zc[:, :], w1x_sb[:, k, :], xbf[:, k, :],
                    start=(k == 0), stop=False,
                )
            for k in range(CB):
                nc.tensor.matmul(
                    zc[:, :], w1s_sb[:, k, :], std[:, k, :],
                    start=False, stop=(k == CB - 1),
                )
            ym = psum.tile([P, TC], F32, tag="ym")
            for k in range(CB):
                nc.tensor.matmul(
                    ym[:, :], w1m_sb[:, k, :], xt[:, k, :],
                    start=(k == 0), stop=(k == CB - 1),
                )

            # mean-feature: scan(ym; rho_c) directly (column scaling commutes)
            zms = const.tile([P, TC], BF16, tag="zms")
            init = 0.0 if ch == 0 else ymcar[:, :]
            nc.vector.tensor_tensor_scan(
                zms[:, :], ym[:, :], rho2d, init, ALU.add, ALU.mult
            )
            nc.vector.tensor_copy(ymcar[:], zms[:, TC - 1 : TC])
            nc.vector.tensor_add(z_sb[:, t0 : t0 + TC], zc[:, :], zms[:, :])

        # ================= PHASE 2 =================
        nc.scalar.activation(z_sb[:], z_sb[:], ACT.Tanh, bias=b1_sb[:, 0:1], scale=1.0)

        ebf_row = dbl.tile([1, T], BF16, tag="sqx")
        for j in range(T // TC):
            lg = psum.tile([1, TC], F32, tag="zc")
            nc.tensor.matmul(
                lg[:, :], w2_sb[:, :], z_sb[:, j * TC : (j + 1) * TC],
                start=True, stop=True,
            )
            nc.scalar.activation(
                ebf_row[:, j * TC : (j + 1) * TC], lg[:, :], ACT.Exp,
                bias=b2_sb[:, 0:1], scale=1.0,
            )
        # ---- folded Z / reciprocal / ratio pipeline ([128, FOLD]) ----
        scrb_r = scrb_d.rearrange("o (p f) -> (o p) f", p=P)
        scr_r = scr_d.rearrange("o (p f) -> (o p) f", p=P)
        scrb2_r = scrb2_d.rearrange("o (p f) -> (o p) f", p=P)
        nc.sync.dma_start(scrb_d.ap(), ebf_row[:, :])
        efold = const.tile([P, FOLD], BF16, tag="efold")
        nc.sync.dma_start(efold[:], scrb_r)
        zloc = const.tile([P, FOLD], F32, tag="zloc")
        nc.vector.tensor_tensor_scan(
            zloc[:, :], efold[:, :], efold[:, :], 0.0, ALU.add, ALU.bypass
        )
        offp = psbc.tile([P, 1], F32, tag="bc")
        nc.tensor.matmul(
            offp[:, :], lstrict_sb[:, :], zloc[:, FOLD - 1 : FOLD],
            start=True, stop=True,
        )
        offc = const.tile([P, 1], F32, tag="offc")
        nc.scalar.copy(offc[:], offp[:])
        zfold = const.tile([P, FOLD], F32, tag="zfold")
        nc.vector.tensor_scalar(zfold[:], zloc[:], offc[:, 0:1], None, ALU.add)
        rzfold = const.tile([P, FOLD], F32, tag="rzfold")
        nc.vector.reciprocal(rzfold[:], zfold[:])
        # Z_prev (shift by one, seam via sub-diagonal matmul), Z_prev[0] = 1
        seamp = psbc.tile([P, 1], F32, tag="bc")
        nc.tensor.matmul(
            seamp[:, :], ssub_sb[:, :], zfold[:, FOLD - 1 : FOLD],
            start=True, stop=True,
        )
        zpf = const.tile([P, FOLD], F32, tag="zpf")
        nc.vector.tensor_copy(zpf[:, 1:FOLD], zfold[:, 0 : FOLD - 1])
        nc.scalar.copy(zpf[:, 0:1], seamp[:, :])
        nc.vector.memset(zpf[0:1, 0:1], 1.0)
        rzpf = const.tile([P, FOLD], F32, tag="rzpf")
        nc.vector.reciprocal(rzpf[:], zpf[:])
        rhozf = const.tile([P, FOLD], F32, tag="rhozf")
        nc.vector.tensor_mul(rhozf[:], zpf[:], rzfold[:])
        etf = const.tile([P, FOLD], BF16, tag="etf")
        nc.vector.tensor_mul(etf[:], efold[:], rzpf[:])
        # unfold to DRAM, broadcast back into the big tables
        nc.sync.dma_start(scr_r, rhozf[:])
        nc.sync.dma_start(scrb2_r, etf[:])
        etB = const.tile([P, T], BF16, tag="tblA")  # reuse mrcB slot
        nc.sync.dma_start(etB[:], scrb2_d.ap().broadcast_to([P, T]))
        nc.sync.dma_start(rhoB[:], scr_d.ap().broadcast_to([P, T]))

        # ================= PHASE 3 =================
        for ch in range(NCH):
            t0 = ch * TC

            xbf = xpool.tile([P, CB, TC], BF16, tag="xbf")
            nc.gpsimd.memset(xbf[:, :, 0:1], 0.0)
            nc.gpsimd.dma_start(xbf[:], x_r[:, :, t0 : t0 + TC])

            rho2d = rhoB[:, t0 : t0 + TC]

            gt = dbl.tile([P, CB, TC], BF16, tag="xt")
            nc.vector.tensor_mul(gt[:], xbf[:], bcslc(etB, t0))
            wm = hot.tile([P, CB, TC], BF16, tag="mean")
            for k in range(CB):
                init = 0.0 if ch == 0 else wmcar[:, k, :]
                nc.vector.tensor_tensor_scan(
                    wm[:, k, :], gt[:, k, :], rho2d, init, ALU.add, ALU.mult
                )
            nc.vector.tensor_copy(wmcar[:], wm[:, :, TC - 1 : TC])

            # fm partial sums via ScalarE copy+accumulate (dummy out goes to
            # a double-buffered slot so it doesn't WAR-block next chunk's gt)
            fmdum = hot.tile([P, CB, TC], BF16, tag="b")
            for k in range(CB):
                nc.scalar.activation(
                    fmdum[:, k, :], wm[:, k, :], ACT.Copy,
                    accum_out=fm_stage[:, k : k + 1],
                )
            nc.vector.tensor_add(fm_acc[:], fm_acc[:], fm_stage[:])

            d = dbl.tile([P, CB, TC], BF16, tag="sqx")
            nc.vector.tensor_sub(d[:], xbf[:], wm[:])
            dd = hot.tile([P, CB, TC], BF16, tag="b")
            nc.scalar.activation(dd[:], d[:], ACT.Square)
            nc.vector.tensor_mul(dd[:], dd[:], bcslc(etB, t0))  # e~ * d^2
            wvar = stdp.tile([P, CB, TC], BF16, tag="std")
            for k in range(CB):
                init = 0.0 if ch == 0 else wvcar[:, k, :]
                nc.vector.tensor_tensor_scan(
                    wvar[:, k, :], dd[:, k, :], rho2d, init, ALU.add, ALU.mult
                )
            nc.vector.tensor_copy(wvcar[:], wvar[:, :, TC - 1 : TC])

            wstd = hot.tile([P, CB, TC], BF16, tag="mean")  # dummy out
            for k in range(CB):
                nc.scalar.activation(
                    wstd[:, k, :], wvar[:, k, :], ACT.Sqrt,
                    accum_out=fs_stage[:, k : k + 1],
                )
            nc.vector.tensor_add(fs_acc[:], fs_acc[:], fs_stage[:])

        # ================= FINALIZE =================
        nc.vector.tensor_scalar(fm_acc[:], fm_acc[:], FW, None, ALU.mult)
        nc.vector.tensor_scalar(fs_acc[:], fs_acc[:], FW, None, ALU.mult)
        nc.sync.dma_start(out_r[0], fm_acc[:])
        nc.sync.dma_start(out_r[1], fs_acc[:])

    nc.finalize()
    return nc


def _get_program():
    if "nc" not in _CACHE:
        _CACHE["nc"] = build_program()
    return _CACHE["nc"]


def host_tables(ln, Tdim):
    """Per-sample tables: m/count_prev (bf16), count_prev (bf16),
    count_prev/count (f32)."""
    t = np.arange(Tdim)
    m = (t < ln).astype(np.float64)
    count = np.clip(np.cumsum(m), 1.0, None)
    cprev = np.concatenate([[1.0], count[:-1]])
    mrc = (m / cprev).astype(BF).reshape(1, Tdim)
    cp = cprev.astype(BF).reshape(1, Tdim)
    rhoc = (cprev / count).astype(np.float32).reshape(1, Tdim)
    return mrc, cp, rhoc


_LSTRICT = (np.tril(np.ones((P, P)), -1) - np.tril(np.ones((P, P)), -1).T * 0).astype(np.float32).T.copy()
_SSUB = np.zeros((P, P), np.float32)
for _i in range(1, P):
    _SSUB[_i - 1, _i] = 1.0


def make_in_map(xb, ln, W1, b1, W2, b2, Cdim, Tdim):
    mrc, cp, rhoc = host_tables(ln, Tdim)
    return {
        "lstrict": _LSTRICT,
        "ssub": _SSUB,
        "x": np.ascontiguousarray(xb),
        "mrcrow": mrc,
        "cprow": cp,
        "rhocrow": rhoc,
        "w1xT": np.ascontiguousarray(W1[:, 0:Cdim].T).astype(BF),
        "w1mT": np.ascontiguousarray(W1[:, Cdim : 2 * Cdim].T).astype(BF),
        "w1sT": np.ascontiguousarray(W1[:, 2 * Cdim : 3 * Cdim].T).astype(BF),
        "w2col": np.ascontiguousarray(W2.T).astype(BF),
        "b1col": b1.reshape(A, 1).astype(np.float32),
        "b2val": b2.reshape(1, 1).astype(np.float32),
    }


def kernel(x, lengths, W1, b1, W2, b2):
    x = np.asarray(x, dtype=np.float32)
    lengths = np.asarray(lengths)
    W1 = np.asarray(W1, dtype=np.float32)
    b1 = np.asarray(b1, dtype=np.float32)
    W2 = np.asarray(W2, dtype=np.float32)
    b2 = np.asarray(b2, dtype=np.float32)

    nc = _get_program()
    in_maps = [
        make_in_map(x[b], int(lengths[b]), W1, b1, W2, b2, C, T) for b in range(B)
    ]

    import os

    trace = bool(os.environ.get("BASS_KERNEL_TRACE"))
    try:
        res = run_bass_kernel_spmd(nc, in_maps, core_ids=list(range(B)), trace=trace)
    except Exception:
        # transient device errors have been observed; retry once
        import time as _time

        _time.sleep(2.0)
        res = run_bass_kernel_spmd(nc, in_maps, core_ids=list(range(B)), trace=trace)
    _CACHE["exec_time_ns"] = getattr(res, "exec_time_ns", None)
    _CACHE["results_obj"] = res

    outs = []
    for b in range(B):
        o = np.asarray(res.results[b]["out"], dtype=np.float32)
        outs.append(np.concatenate([o[0].reshape(C), o[1].reshape(C)]))
    return np.stack(outs).astype(np.float32)


# revision 31
# speedup vs baseline: 1.1373x; 1.0152x over previous
"""Causal attentive statistics pooling — Trainium2 Bass kernel (v2).

Strategy (hardcoded for B=8, C=1536, T=4096, A=128, 8 cores):
  - Data-parallel over batch: one sample per NeuronCore.
  - Layout: channels on partitions (12 blocks of 128), time on the free axis.
    Bulk elementwise in bf16 (DVE 2x), prefix ops via tensor_tensor_scan.
  - Key trick: the running mean / running normalized sums are computed with a
    single ratio-recurrence scan  state_t = (d0_t + state_{t-1}) * rho_t
    where rho = count_{t-1}/count_t (resp. Z_{t-1}/Z_t) is an fp32 broadcast
    table.  This emits mean, E[x^2], weighted-mean, and weighted-var directly
    from the scan with no separate [C,T]-sized multiply passes.
  - The causal-mean attention term uses scan(W1m @ (x*m/count_prev)) (matmul
    and column-scaled prefix-sum commute), so mean is never an input to PE.
  - Squares run on ScalarE; sqrt with fused row-sum accumulators produces the
    final std sums; weighted-mean sums come from ScalarE copy+accumulate.
"""

import sys

sys.path.insert(0, "/opt/trn_rl_repo")

from contextlib import ExitStack

import ml_dtypes
import numpy as np

import concourse.bass as bass
import concourse.tile as tile
from concourse import bacc
from concourse import mybir
from concourse.bass_utils import run_bass_kernel_spmd

B, C, T, A = 8, 1536, 4096, 128
P = 128
CB = C // P  # channel blocks
TC = 512  # time chunk
NCH = T // TC
EPS = 1e-12
FW = float(1.0 / (T + EPS))

F32 = mybir.dt.float32
BF16 = mybir.dt.bfloat16
ALU = mybir.AluOpType
ACT = mybir.ActivationFunctionType
BF = ml_dtypes.bfloat16

_CACHE = {}


def build_program():
    FOLD = T // P
    nc = bacc.Bacc("TRN2", target_bir_lowering=False, debug=False)
    scr_d = nc.dram_tensor("zscratch", [1, T], F32)
    scrb_d = nc.dram_tensor("escratch", [1, T], BF16)
    scrb2_d = nc.dram_tensor("etscratch", [1, T], BF16)

    x_d = nc.dram_tensor("x", [C, T], F32, kind="ExternalInput")
    mrc_d = nc.dram_tensor("mrcrow", [1, T], BF16, kind="ExternalInput")
    cp_d = nc.dram_tensor("cprow", [1, T], BF16, kind="ExternalInput")
    rhoc_d = nc.dram_tensor("rhocrow", [1, T], F32, kind="ExternalInput")
    lstrict_d = nc.dram_tensor("lstrict", [P, P], F32, kind="ExternalInput")
    ssub_d = nc.dram_tensor("ssub", [P, P], F32, kind="ExternalInput")
    w1x_d = nc.dram_tensor("w1xT", [C, A], BF16, kind="ExternalInput")
    w1m_d = nc.dram_tensor("w1mT", [C, A], BF16, kind="ExternalInput")
    w1s_d = nc.dram_tensor("w1sT", [C, A], BF16, kind="ExternalInput")
    w2_d = nc.dram_tensor("w2col", [A, 1], BF16, kind="ExternalInput")
    b1_d = nc.dram_tensor("b1col", [A, 1], F32, kind="ExternalInput")
    b2_d = nc.dram_tensor("b2val", [1, 1], F32, kind="ExternalInput")
    out_d = nc.dram_tensor("out", [2, CB, P], F32, kind="ExternalOutput")

    x_r = x_d.rearrange("(k p) t -> p k t", p=P)
    out_r = out_d.rearrange("s k p -> s p k")

    with tile.TileContext(nc) as tc, ExitStack() as ctx:
        const = ctx.enter_context(tc.tile_pool(name="const", bufs=1))
        xpool = ctx.enter_context(tc.tile_pool(name="xpool", bufs=3))
        dbl = ctx.enter_context(tc.tile_pool(name="dbl", bufs=1))
        stdp = ctx.enter_context(tc.tile_pool(name="stdp", bufs=2))
        hot = ctx.enter_context(tc.tile_pool(name="hot", bufs=2))
        psum = ctx.enter_context(tc.tile_pool(name="psum", bufs=2, space="PSUM"))
        psbc = ctx.enter_context(tc.tile_pool(name="psbc", bufs=2, space="PSUM"))

        def bcslc(tbl, t0):
            return (
                tbl[:, t0 : t0 + TC]
                .rearrange("p (o t) -> p o t", o=1)
                .broadcast_to([P, CB, TC])
            )

        # broadcast tables: mrcB/cpB bf16; rhoB f32 (shared phase1/phase3)
        mrcB = const.tile([P, T], BF16, tag="tblA")  # m/count_prev, later e~
        cpB = const.tile([P, T], BF16, tag="tblB")  # count_prev
        rhoB = const.tile([P, T], F32, tag="tblR")  # rho_c, later rho_z

        # host tables broadcast straight from DRAM (partition-stride-0 DMA)
        nc.sync.dma_start(mrcB[:], mrc_d.ap().broadcast_to([P, T]))
        nc.sync.dma_start(cpB[:], cp_d.ap().broadcast_to([P, T]))
        nc.sync.dma_start(rhoB[:], rhoc_d.ap().broadcast_to([P, T]))
        # ---- weights / host tables ----
        w1x_sb = const.tile([P, CB, A], BF16)
        w1m_sb = const.tile([P, CB, A], BF16)
        w1s_sb = const.tile([P, CB, A], BF16)
        nc.sync.dma_start(w1x_sb[:], w1x_d.rearrange("(k p) m -> p k m", p=P))
        nc.sync.dma_start(w1m_sb[:], w1m_d.rearrange("(k p) m -> p k m", p=P))
        nc.sync.dma_start(w1s_sb[:], w1s_d.rearrange("(k p) m -> p k m", p=P))
        w2_sb = const.tile([A, 1], BF16)
        b1_sb = const.tile([A, 1], F32)
        b2_sb = const.tile([1, 1], F32)
        nc.sync.dma_start(w2_sb[:], w2_d.ap())
        nc.sync.dma_start(b1_sb[:], b1_d.ap())
        nc.sync.dma_start(b2_sb[:], b2_d.ap())
        lstrict_sb = const.tile([P, P], F32)
        ssub_sb = const.tile([P, P], F32)
        nc.sync.dma_start(lstrict_sb[:], lstrict_d.ap())
        nc.sync.dma_start(ssub_sb[:], ssub_d.ap())


        # carries and accumulators
        meancar = const.tile([P, CB, 1], F32)
        bcar = const.tile([P, CB, 1], F32)
        wmcar = const.tile([P, CB, 1], F32)
        wvcar = const.tile([P, CB, 1], F32)
        ymcar = const.tile([P, 1], F32)
        fm_acc = const.tile([P, CB], F32)
        fs_acc = const.tile([P, CB], F32)
        fm_stage = const.tile([P, CB], F32)
        fs_stage = const.tile([P, CB], F32)
        nc.vector.memset(fs_acc[:], 0.0)
        nc.vector.memset(fm_acc[:], 0.0)

        z_sb = const.tile([P, T], BF16, tag="z")

        # ================= PHASE 1 =================
        for ch in range(NCH):
            t0 = ch * TC

            xbf = xpool.tile([P, CB, TC], BF16, tag="xbf")
            # tiny same-engine write absorbs WAR waits (DMA sync-wait limit)
            nc.gpsimd.memset(xbf[:, :, 0:1], 0.0)
            nc.gpsimd.dma_start(xbf[:], x_r[:, :, t0 : t0 + TC])

            # xt = x * m / count_prev  (mask folded into the table)
            xt = dbl.tile([P, CB, TC], BF16, tag="xt")
            nc.vector.tensor_mul(xt[:], xbf[:], bcslc(mrcB, t0))
            # xxt = xt^2 * count_prev = x^2 m / count_prev
            sqx = dbl.tile([P, CB, TC], BF16, tag="sqx")
            nc.scalar.activation(sqx[:], xt[:], ACT.Square)
            nc.vector.tensor_mul(sqx[:], sqx[:], bcslc(cpB, t0))

            rho2d = rhoB[:, t0 : t0 + TC]
            mean = hot.tile([P, CB, TC], BF16, tag="mean")
            bm2 = hot.tile([P, CB, TC], BF16, tag="b")
            for k in range(CB):
                init = 0.0 if ch == 0 else meancar[:, k, :]
                nc.vector.tensor_tensor_scan(
                    mean[:, k, :], xt[:, k, :], rho2d, init, ALU.add, ALU.mult
                )
            nc.vector.tensor_copy(meancar[:], mean[:, :, TC - 1 : TC])
            for k in range(CB):
                init = 0.0 if ch == 0 else bcar[:, k, :]
                nc.vector.tensor_tensor_scan(
                    bm2[:, k, :], sqx[:, k, :], rho2d, init, ALU.add, ALU.mult
                )
            nc.vector.tensor_copy(bcar[:], bm2[:, :, TC - 1 : TC])

            # var = clamp(b - mean^2), std = sqrt
            mm = dbl.tile([P, CB, TC], BF16, tag="sqx")  # reuse
            nc.scalar.activation(mm[:], mean[:], ACT.Square)
            nc.vector.tensor_sub(bm2[:], bm2[:], mm[:])
            nc.vector.tensor_scalar(bm2[:], bm2[:], EPS, None, ALU.max)
            std = stdp.tile([P, CB, TC], BF16, tag="std")
            nc.scalar.activation(std[:], bm2[:], ACT.Sqrt)

            # PE: zc = W1x @ x + W1s @ std ; ym = W1m @ xt
            zc = psum.tile([P, TC], F32, tag="zc")
            for k in range(CB):
                nc.tensor.matmul(
                    zc[:, :], w1x_sb[:, k, :], xbf[:, k, :],
                    start=(k == 0), stop=False,
                )
            for k in range(CB):
                nc.tensor.matmul(
                    zc[:, :], w1s_sb[:, k, :], std[:, k, :],
                    start=False, stop=(k == CB - 1),
                )
            ym = psum.tile([P, TC], F32, tag="ym")
            for k in range(CB):
                nc.tensor.matmul(
                    ym[:, :], w1m_sb[:, k, :], xt[:, k, :],
                    start=(k == 0), stop=(k == CB - 1),
                )

            # mean-feature: scan(ym; rho_c) directly (column scaling commutes)
            zms = const.tile([P, TC], BF16, tag="zms")
            init = 0.0 if ch == 0 else ymcar[:, :]
            nc.vector.tensor_tensor_scan(
                zms[:, :], ym[:, :], rho2d, init, ALU.add, ALU.mult
            )
            nc.vector.tensor_copy(ymcar[:], zms[:, TC - 1 : TC])
            nc.vector.tensor_add(z_sb[:, t0 : t0 + TC], zc[:, :], zms[:, :])

        # ================= PHASE 2 =================
        nc.scalar.activation(z_sb[:], z_sb[:], ACT.Tanh, bias=b1_sb[:, 0:1], scale=1.0)

        ebf_row = dbl.tile([1, T], BF16, tag="sqx")
        for j in range(T // TC):
            lg = psum.tile([1, TC], F32, tag="zc")
            nc.tensor.matmul(
                lg[:, :], w2_sb[:, :], z_sb[:, j * TC : (j + 1) * TC],
                start=True, stop=True,
            )
            nc.scalar.activation(
                ebf_row[:, j * TC : (j + 1) * TC], lg[:, :], ACT.Exp,
                bias=b2_sb[:, 0:1], scale=1.0,
            )
        # ---- folded Z / reciprocal / ratio pipeline ([128, FOLD]) ----
        scrb_r = scrb_d.rearrange("o (p f) -> (o p) f", p=P)
        scr_r = scr_d.rearrange("o (p f) -> (o p) f", p=P)
        scrb2_r = scrb2_d.rearrange("o (p f) -> (o p) f", p=P)
        nc.sync.dma_start(scrb_d.ap(), ebf_row[:, :])
        efold = const.tile([P, FOLD], BF16, tag="efold")
        nc.sync.dma_start(efold[:], scrb_r)
        zloc = const.tile([P, FOLD], F32, tag="zloc")
        nc.vector.tensor_tensor_scan(
            zloc[:, :], efold[:, :], efold[:, :], 0.0, ALU.add, ALU.bypass
        )
        offp = psbc.tile([P, 1], F32, tag="bc")
        nc.tensor.matmul(
            offp[:, :], lstrict_sb[:, :], zloc[:, FOLD - 1 : FOLD],
            start=True, stop=True,
        )
        offc = const.tile([P, 1], F32, tag="offc")
        nc.scalar.copy(offc[:], offp[:])
        zfold = const.tile([P, FOLD], F32, tag="zfold")
        nc.vector.tensor_scalar(zfold[:], zloc[:], offc[:, 0:1], None, ALU.add)
        rzfold = const.tile([P, FOLD], F32, tag="rzfold")
        nc.vector.reciprocal(rzfold[:], zfold[:])
        # Z_prev (shift by one, seam via sub-diagonal matmul), Z_prev[0] = 1
        seamp = psbc.tile([P, 1], F32, tag="bc")
        nc.tensor.matmul(
            seamp[:, :], ssub_sb[:, :], zfold[:, FOLD - 1 : FOLD],
            start=True, stop=True,
        )
        zpf = const.tile([P, FOLD], F32, tag="zpf")
        nc.vector.tensor_copy(zpf[:, 1:FOLD], zfold[:, 0 : FOLD - 1])
        nc.scalar.copy(zpf[:, 0:1], seamp[:, :])
        nc.vector.memset(zpf[0:1, 0:1], 1.0)
        rzpf = const.tile([P, FOLD], F32, tag="rzpf")
        nc.vector.reciprocal(rzpf[:], zpf[:])
        rhozf = const.tile([P, FOLD], F32, tag="rhozf")
        nc.vector.tensor_mul(rhozf[:], zpf[:], rzfold[:])
        etf = const.tile([P, FOLD], BF16, tag="etf")
        nc.vector.tensor_mul(etf[:], efold[:], rzpf[:])
        # unfold to DRAM, broadcast back into the big tables
        # two independent unfold+broadcast chains on separate HWDGE queues
        nc.sync.dma_start(scr_r, rhozf[:])
        nc.scalar.dma_start(scrb2_r, etf[:])
        etB = const.tile([P, T], BF16, tag="tblA")  # reuse mrcB slot
        nc.scalar.dma_start(etB[:], scrb2_d.ap().broadcast_to([P, T]))
        nc.sync.dma_start(rhoB[:], scr_d.ap().broadcast_to([P, T]))

        # ================= PHASE 3 =================
        for ch in range(NCH):
            t0 = ch * TC

            xbf = xpool.tile([P, CB, TC], BF16, tag="xbf")
            nc.gpsimd.memset(xbf[:, :, 0:1], 0.0)
            nc.gpsimd.dma_start(xbf[:], x_r[:, :, t0 : t0 + TC])

            rho2d = rhoB[:, t0 : t0 + TC]

            gt = dbl.tile([P, CB, TC], BF16, tag="xt")
            nc.vector.tensor_mul(gt[:], xbf[:], bcslc(etB, t0))
            wm = hot.tile([P, CB, TC], BF16, tag="mean")
            for k in range(CB):
                init = 0.0 if ch == 0 else wmcar[:, k, :]
                nc.vector.tensor_tensor_scan(
                    wm[:, k, :], gt[:, k, :], rho2d, init, ALU.add, ALU.mult
                )
            nc.vector.tensor_copy(wmcar[:], wm[:, :, TC - 1 : TC])

            # fm partial sums via ScalarE copy+accumulate (dummy out goes to
            # a double-buffered slot so it doesn't WAR-block next chunk's gt)
            fmdum = hot.tile([P, CB, TC], BF16, tag="b")
            for k in range(CB):
                nc.scalar.activation(
                    fmdum[:, k, :], wm[:, k, :], ACT.Copy,
                    accum_out=fm_stage[:, k : k + 1],
                )
            nc.vector.tensor_add(fm_acc[:], fm_acc[:], fm_stage[:])

            d = dbl.tile([P, CB, TC], BF16, tag="sqx")
            nc.vector.tensor_sub(d[:], xbf[:], wm[:])
            dd = hot.tile([P, CB, TC], BF16, tag="b")
            nc.scalar.activation(dd[:], d[:], ACT.Square)
            nc.vector.tensor_mul(dd[:], dd[:], bcslc(etB, t0))  # e~ * d^2
            wvar = stdp.tile([P, CB, TC], BF16, tag="std")
            for k in range(CB):
                init = 0.0 if ch == 0 else wvcar[:, k, :]
                nc.vector.tensor_tensor_scan(
                    wvar[:, k, :], dd[:, k, :], rho2d, init, ALU.add, ALU.mult
                )
            nc.vector.tensor_copy(wvcar[:], wvar[:, :, TC - 1 : TC])

            wstd = hot.tile([P, CB, TC], BF16, tag="mean")  # dummy out
            for k in range(CB):
                nc.scalar.activation(
                    wstd[:, k, :], wvar[:, k, :], ACT.Sqrt,
                    accum_out=fs_stage[:, k : k + 1],
                )
            nc.vector.tensor_add(fs_acc[:], fs_acc[:], fs_stage[:])

        # ================= FINALIZE =================
        nc.vector.tensor_scalar(fm_acc[:], fm_acc[:], FW, None, ALU.mult)
        nc.vector.tensor_scalar(fs_acc[:], fs_acc[:], FW, None, ALU.mult)
        nc.sync.dma_start(out_r[0], fm_acc[:])
        nc.sync.dma_start(out_r[1], fs_acc[:])

    nc.finalize()
    return nc


def _get_program():
    if "nc" not in _CACHE:
        _CACHE["nc"] = build_program()
    return _CACHE["nc"]


def host_tables(ln, Tdim):
    """Per-sample tables: m/count_prev (bf16), count_prev (bf16),
    count_prev/count (f32)."""
    t = np.arange(Tdim)
    m = (t < ln).astype(np.float64)
    count = np.clip(np.cumsum(m), 1.0, None)
    cprev = np.concatenate([[1.0], count[:-1]])
    mrc = (m / cprev).astype(BF).reshape(1, Tdim)
    cp = cprev.astype(BF).reshape(1, Tdim)
    rhoc = (cprev / count).astype(np.float32).reshape(1, Tdim)
    return mrc, cp, rhoc


_LSTRICT = (np.tril(np.ones((P, P)), -1) - np.tril(np.ones((P, P)), -1).T * 0).astype(np.float32).T.copy()
_SSUB = np.zeros((P, P), np.float32)
for _i in range(1, P):
    _SSUB[_i - 1, _i] = 1.0


def make_in_map(xb, ln, W1, b1, W2, b2, Cdim, Tdim):
    mrc, cp, rhoc = host_tables(ln, Tdim)
    return {
        "lstrict": _LSTRICT,
        "ssub": _SSUB,
        "x": np.ascontiguousarray(xb),
        "mrcrow": mrc,
        "cprow": cp,
        "rhocrow": rhoc,
        "w1xT": np.ascontiguousarray(W1[:, 0:Cdim].T).astype(BF),
        "w1mT": np.ascontiguousarray(W1[:, Cdim : 2 * Cdim].T).astype(BF),
        "w1sT": np.ascontiguousarray(W1[:, 2 * Cdim : 3 * Cdim].T).astype(BF),
        "w2col": np.ascontiguousarray(W2.T).astype(BF),
        "b1col": b1.reshape(A, 1).astype(np.float32),
        "b2val": b2.reshape(1, 1).astype(np.float32),
    }


def kernel(x, lengths, W1, b1, W2, b2):
    x = np.asarray(x, dtype=np.float32)
    lengths = np.asarray(lengths)
    W1 = np.asarray(W1, dtype=np.float32)
    b1 = np.asarray(b1, dtype=np.float32)
    W2 = np.asarray(W2, dtype=np.float32)
    b2 = np.asarray(b2, dtype=np.float32)

    nc = _get_program()
    in_maps = [
        make_in_map(x[b], int(lengths[b]), W1, b1, W2, b2, C, T) for b in range(B)
    ]

    import os

    trace = bool(os.environ.get("BASS_KERNEL_TRACE"))
    try:
        res = run_bass_kernel_spmd(nc, in_maps, core_ids=list(range(B)), trace=trace)
    except Exception:
        # transient device errors have been observed; retry once
        import time as _time

        _time.sleep(2.0)
        res = run_bass_kernel_spmd(nc, in_maps, core_ids=list(range(B)), trace=trace)
    _CACHE["exec_time_ns"] = getattr(res, "exec_time_ns", None)
    _CACHE["results_obj"] = res

    outs = []
    for b in range(B):
        o = np.asarray(res.results[b]["out"], dtype=np.float32)
        outs.append(np.concatenate([o[0].reshape(C), o[1].reshape(C)]))
    return np.stack(outs).astype(np.float32)
